# revision 1
# baseline (speedup 1.0000x reference)
"""EqualizedFocalLoss kernel for 8 Trainium2 NeuronCores.

Strategy
--------
The loss is dominated by the focal reduction over pred/gt ([32,15,256,256]
f32 each, ~125.8 MB per tensor).  That part is memory-bound and runs on
device, data-parallel over batch (4 batches per core):

    device S = sum_c (gamma_c/2) * sum_{b,h,w} ln(1-p+eps) * p^gamma_c * (1-gt)^4

computed at the *unmodified* pred.  Everything index-sized — the [B,K]
gather + smooth-L1, the multiplicative scatter (which touches at most
B*K = 16000 positions), the correction of the focal sum at those
positions, loss0, and num_pos handling — is exact fp64 host math.

Per core the device program streams 15 channel-tiles of [128, 4*512]
(fp32 in HBM, fp16 intermediates — fp16 keeps DVE's 2x mode and avoids the
bf16 correlated-rounding bias of the (1-gt)^4 chain):
  ACT   : lq = Ln(1-p); pg = Exp(g*Ln(p+eps) + ln(g/2)), or Square for
          gamma in {2, 3} (all three live in one activation-table set)
  GPSIMD: omg = 1 - gt
  DVE   : w2 = omg^2, nw = w2^2, t1 = lq*pg, t2 = t1*nw   (fp16, 2x mode)
  PE    : ones[128,1].T @ t2-chunks accumulated into one PSUM [1,512] row
          (gamma/2 folded into the Exp bias or the lhsT constant)
Exp-path channels run first and the cheap Square-path channels last, with
the final two channels split into per-plane chunks, so the post-DMA tail is
short; the kernel is DMA-bound at ~87us of 100us predicted total.
The PSUM row is copied out once; the host sums the 512 partials.
"""

import math

import numpy as np

B, NCLS, H, W = 32, 15, 256, 256
K, CREG = 500, 2
N_CORES = 8
BPC = B // N_CORES  # batches per core
HW = H * W
P = 128
F = HW // P  # 512
FREE = BPC * F  # 2048
EPS = 1e-12

GAMMAS = np.array(
    [2.7, 2.1, 2.4, 2.0, 3.0, 2.9, 3.0, 2.5, 2.1, 2.6, 2.0, 2.1, 2.7, 2.4, 2.2],
    dtype=np.float64,
)

_CACHE = {}


def _patch_act_tables(bacc, mybir):
    """Force Bacc's table-load chooser to use natural_log_exp_and_others for
    Ln/Exp/Square so the kernel needs exactly one ACT_TABLE_LOAD instead of
    thrashing between per-function sets.  Only set *membership* is edited —
    dict order (the act_func_set_id mapping) is preserved."""
    if getattr(bacc, "_efl_act_tables_patched", False):
        return
    orig = bacc.get_activation_tables
    ACT = mybir.ActivationFunctionType
    targets = {ACT.Ln, ACT.Exp, ACT.Square}
    keep = "natural_log_exp_and_others"

    def patched(arch):
        tabs = {k: set(v) for k, v in orig(arch).items()}
        if keep in tabs:
            prot = tabs[keep] & targets
            for name, s in tabs.items():
                if name != keep:
                    s -= prot
        return tabs

    bacc.get_activation_tables = patched
    bacc._efl_act_tables_patched = True


def _build_bass():
    import concourse.tile as tile
    from concourse import bacc, mybir

    _patch_act_tables(bacc, mybir)
    nc = bacc.Bacc()
    pred = nc.dram_tensor(
        "pred", [BPC, NCLS, HW], mybir.dt.float32, kind="ExternalInput"
    )
    gt = nc.dram_tensor("gt", [BPC, NCLS, HW], mybir.dt.float32, kind="ExternalInput")
    out = nc.dram_tensor("out", [1, F], mybir.dt.float32, kind="ExternalOutput")

    fdt = mybir.dt.float32
    bdt = mybir.dt.float16
    ALU = mybir.AluOpType
    ACT = mybir.ActivationFunctionType

    # Register activation-bias constants the same way Bass registers its
    # built-in const APs: memset before an all-engine barrier, so later reads
    # need no semaphore waits (the AC instruction has very few sync slots).
    _eng = [nc.gpsimd, nc.vector]

    def register_const(value):
        key = (fdt, value)
        if key in nc.const_aps.aps:
            return
        t = nc.alloc_sbuf_tensor(f"kconst-{len(nc.const_aps.aps)}", [P, 1], fdt)
        _eng[len(nc.const_aps.aps) % len(_eng)].memset(t.ap(), value)
        nc.const_aps.aps[key] = t.ap()

    register_const(EPS)
    for _g in sorted(set(GAMMAS.tolist())):
        register_const(math.log(_g / 2.0))
    nc.all_engine_barrier()

    with tile.TileContext(nc) as tc:
        with (
            tc.tile_pool(name="io", bufs=4) as io_pool,
            tc.tile_pool(name="mid", bufs=3) as mid_pool,
            tc.tile_pool(name="fix", bufs=1) as fix_pool,
            tc.tile_pool(name="psum", bufs=1, space="PSUM") as psum_pool,
        ):
            ones = fix_pool.tile([P, 1], bdt)
            nc.vector.memset(ones, 1.0)
            # gamma/2 = 1.5 for the gamma==3 channels, exact in fp16
            ones15 = fix_pool.tile([P, 1], bdt, tag="ones15")
            nc.vector.memset(ones15, 1.5)
            psum_f = psum_pool.tile([1, F], mybir.dt.float32)

            # Warm the Ln/Exp activation tables on dependency-free dummy ops so
            # walrus attaches ACT_TABLE_LOAD to an instruction with no waits.
            warm = fix_pool.tile([P, 1], fdt, tag="warm")
            const1 = nc.const_aps.tensor(1.0, (P, 1))
            nc.scalar.activation(out=warm, in_=const1, func=ACT.Ln, bias=1.0)
            nc.scalar.activation(out=warm, in_=const1, func=ACT.Exp, bias=0.0)

            pred_r = pred[:].rearrange("b c (p f) -> c p b f", p=P)
            gt_r = gt[:].rearrange("b c (p f) -> c p b f", p=P)

            n_chunks = FREE // F
            # Process the Exp-path channels (3 ACT passes, slower than the
            # 5.83us/channel DMA rate) first and the cheap Square-path
            # channels (2 ACT passes) last, so ACT drains its backlog before
            # the final tile and the post-DMA tail stays short.
            order = (
                [c for c in range(NCLS) if float(GAMMAS[c]) == 3.0]
                + [c for c in range(NCLS) if float(GAMMAS[c]) not in (2.0, 3.0)]
                + [c for c in range(NCLS) if float(GAMMAS[c]) == 2.0]
            )
            for ci, c in enumerate(order):
                g = float(GAMMAS[c])
                last = ci == NCLS - 1
                tailish = ci >= NCLS - 2
                p_t = io_pool.tile([P, BPC, F], fdt, tag="p")
                g_t = io_pool.tile([P, BPC, F], fdt, tag="g")
                if tailish and not last:
                    # Second-to-last channel: gt lands first as one transfer
                    # (its full-tile omg/w2/nw run early, off the tail);
                    # pred is chunked for the pipelined pred-side below.
                    nc.sync.dma_start(out=g_t, in_=gt_r[c])
                    for j in range(BPC):
                        nc.sync.dma_start(out=p_t[:, j], in_=pred_r[c][:, j])
                elif last:
                    # Final channel: interleave gt/pred per plane so the
                    # chunked chain starts as soon as the first planes land.
                    for j in range(BPC):
                        nc.sync.dma_start(out=g_t[:, j], in_=gt_r[c][:, j])
                        nc.sync.dma_start(out=p_t[:, j], in_=pred_r[c][:, j])
                else:
                    nc.sync.dma_start(out=p_t, in_=pred_r[c])
                    nc.sync.dma_start(out=g_t, in_=gt_r[c])
                p2 = p_t.rearrange("p b f -> p (b f)")
                g2 = g_t.rearrange("p b f -> p (b f)")

                omg = mid_pool.tile([P, FREE], bdt, tag="omg")
                w2 = mid_pool.tile([P, FREE], bdt, tag="w2")
                nw = mid_pool.tile([P, FREE], bdt, tag="nw")
                lq = mid_pool.tile([P, FREE], bdt, tag="lq")
                pg = mid_pool.tile([P, FREE], bdt, tag="pg")
                p2sq = mid_pool.tile([P, FREE], bdt, tag="p2sq")
                lp = mid_pool.tile([P, FREE], fdt, tag="lp")
                t1 = mid_pool.tile([P, FREE], bdt, tag="t1")
                t2 = mid_pool.tile([P, FREE], bdt, tag="t2")
                lhsT = ones15 if g == 3.0 else ones

                # Pipeline the final channel in 4 per-plane chunks so the
                # post-DMA tail is a few small ops instead of full-tile ones.
                chunks = (
                    [slice(j * F, (j + 1) * F) for j in range(BPC)]
                    if tailish
                    else [slice(0, FREE)]
                )
                if tailish and not last:
                    fullsl = slice(0, FREE)
                    nc.gpsimd.tensor_scalar(
                        out=omg[:, fullsl], in0=g2[:, fullsl], scalar1=-1.0,
                        scalar2=1.0, op0=ALU.mult, op1=ALU.add,
                    )
                    nc.vector.tensor_tensor(
                        out=w2[:, fullsl], in0=omg[:, fullsl],
                        in1=omg[:, fullsl], op=ALU.mult,
                    )
                    nc.vector.tensor_tensor(
                        out=nw[:, fullsl], in0=w2[:, fullsl],
                        in1=w2[:, fullsl], op=ALU.mult,
                    )

                for ki, sl in enumerate(chunks):
                    if tailish and not last:
                        pass  # gt side computed at full tile above
                    elif last:
                        # Tail channel: w2 = Square(-gt+1) straight from gt
                        # on ACT (drops Pool's omg from the tail chain).
                        nc.scalar.activation(
                            out=w2[:, sl], in_=g2[:, sl], func=ACT.Square,
                            bias=1.0, scale=-1.0,
                        )
                        nc.vector.tensor_tensor(
                            out=nw[:, sl], in0=w2[:, sl], in1=w2[:, sl],
                            op=ALU.mult,
                        )
                    else:
                        # gt side: omg = 1-gt (Pool; ~2.9us/tile vs DVE
                        # 1.13us, but Pool is far under the DMA floor while
                        # DVE is not)
                        nc.gpsimd.tensor_scalar(
                            out=omg[:, sl], in0=g2[:, sl], scalar1=-1.0,
                            scalar2=1.0, op0=ALU.mult, op1=ALU.add,
                        )
                        nc.vector.tensor_tensor(
                            out=w2[:, sl], in0=omg[:, sl], in1=omg[:, sl],
                            op=ALU.mult,
                        )
                        nc.vector.tensor_tensor(
                            out=nw[:, sl], in0=w2[:, sl], in1=w2[:, sl],
                            op=ALU.mult,
                        )

                    nc.scalar.activation(
                        out=lq[:, sl], in_=p2[:, sl], func=ACT.Ln, bias=1.0,
                        scale=-1.0,
                    )
                    if g == 2.0:
                        # (g/2)*p^g == p^2 exactly.  For the final channel
                        # compute it on idle GPSIMD so ACT only carries lq+w2
                        # per chunk after the last DMA lands; otherwise use
                        # ACT Square (same table set as Ln/Exp).
                        if last:
                            nc.gpsimd.tensor_tensor(
                                out=pg[:, sl], in0=p2[:, sl], in1=p2[:, sl],
                                op=ALU.mult,
                            )
                        else:
                            nc.scalar.activation(
                                out=pg[:, sl], in_=p2[:, sl], func=ACT.Square
                            )
                    elif g == 3.0:
                        # p^2 on ACT, * p on DVE; the g/2 = 1.5 factor rides
                        # on the matmul's lhsT (ones15).
                        nc.scalar.activation(
                            out=p2sq[:, sl], in_=p2[:, sl], func=ACT.Square
                        )
                        nc.vector.tensor_tensor(
                            out=pg[:, sl], in0=p2sq[:, sl], in1=p2[:, sl],
                            op=ALU.mult,
                        )
                    else:
                        nc.scalar.activation(
                            out=lp[:, sl], in_=p2[:, sl], func=ACT.Ln, bias=EPS,
                            scale=1.0,
                        )
                        nc.scalar.activation(
                            out=pg[:, sl], in_=lp[:, sl], func=ACT.Exp,
                            bias=math.log(g / 2.0), scale=g,
                        )

                    t1_eng = nc.gpsimd if (tailish and not last) else nc.vector
                    t1_eng.tensor_tensor(
                        out=t1[:, sl], in0=lq[:, sl], in1=pg[:, sl], op=ALU.mult
                    )
                    nc.vector.tensor_tensor(
                        out=t2[:, sl], in0=t1[:, sl], in1=nw[:, sl], op=ALU.mult
                    )

                    t2v = t2[:, sl].rearrange("p (n f) -> p n f", f=F)
                    nsub = (sl.stop - sl.start) // F
                    for j in range(nsub):
                        nc.tensor.matmul(
                            psum_f,
                            lhsT,
                            t2v[:, j],
                            start=(ci == 0 and ki == 0 and j == 0),
                            stop=(last and ki == len(chunks) - 1 and j == nsub - 1),
                        )

            out_t = fix_pool.tile([1, F], mybir.dt.float32)
            nc.scalar.copy(out=out_t, in_=psum_f)
            nc.sync.dma_start(out=out[:], in_=out_t)

    nc.finalize()
    return nc


def _device_focal_sums(pred, gt):
    """Run the Bass kernel on 8 cores. Returns per-core [1,512] partial-sum
    rows of sum_c (g_c/2)*ln(1-p+eps)*p^g_c*(1-gt)^4 over that core's batches."""
    from concourse.bass_utils import run_bass_kernel_spmd

    if "nc" not in _CACHE:
        _CACHE["nc"] = _build_bass()
    nc = _CACHE["nc"]

    in_maps = []
    for i in range(N_CORES):
        sl = slice(i * BPC, (i + 1) * BPC)
        in_maps.append(
            {
                "pred": np.ascontiguousarray(pred[sl]).reshape(BPC, NCLS, HW),
                "gt": np.ascontiguousarray(gt[sl]).reshape(BPC, NCLS, HW),
            }
        )
    last_exc = None
    for _attempt in range(3):
        try:
            res = run_bass_kernel_spmd(nc, in_maps, core_ids=list(range(N_CORES)))
            return [r["out"] for r in res.results]
        except Exception as e:  # transient NRT_EXEC_UNIT_UNRECOVERABLE on axon
            last_exc = e
            import time as _time

            _time.sleep(5.0)
    raise last_exc


def _host_focal_sum(pred, gt):
    """fp64 host fallback for the bulk focal sum (used only when pred has
    values >= 1.0, where the device's eps-free ln(1-p) would diverge from
    the reference)."""
    S = 0.0
    for c in range(NCLS):
        p = pred[:, c].astype(np.float64)
        gv = gt[:, c].astype(np.float64)
        S += (
            GAMMAS[c]
            * 0.5
            * float(
                np.sum(
                    np.log1p(EPS - p)
                    * np.power(p, GAMMAS[c])
                    * np.power(1.0 - gv, 4)
                )
            )
        )
    return S


def _focal_terms(p, gtv, g):
    """Per-element focal contribution (reference formulas, fp64).
    neg part + pos part; pos only where gt == 1."""
    neg = np.log1p(EPS - p) * np.power(p, g) * np.power(1.0 - gtv, 4)
    pos_mask = gtv == 1.0
    pos = np.where(
        pos_mask, np.log(p + EPS) * np.power(1.0 - p, g), 0.0
    )
    return neg + pos


def kernel(**inputs):
    pred = np.asarray(inputs["pred"], dtype=np.float32)
    gt = np.asarray(inputs["gt"], dtype=np.float32)
    output = np.asarray(inputs["output"], dtype=np.float32)
    mask = np.asarray(inputs["mask"])
    ind = np.asarray(inputs["ind"]).astype(np.int64)
    target = np.asarray(inputs["target"], dtype=np.float32)
    inde = np.asarray(inputs["inde"]).astype(np.int64)

    b, c_out = output.shape[0], output.shape[1]
    k = ind.shape[1]

    # ---- device: bulk focal reduction at unmodified pred -------------------
    if float(pred.max()) >= 1.0:
        # Out-of-distribution input (spec: uniform [0,1)); the device path
        # computes ln(1-p) without eps, which only differs when p >= 1.
        S = _host_focal_sum(pred, gt)
    else:
        parts = _device_focal_sums(pred, gt)
        S = float(sum(np.sum(p.astype(np.float64)) for p in parts))

    # ---- host: gather + smooth-L1 + vals (fp64) ----------------------------
    o2 = output.reshape(b, c_out, -1).astype(np.float64)
    pre = np.stack(
        [np.take_along_axis(o2[:, c, :], ind, axis=1) for c in range(c_out)], axis=2
    )  # [B,K,CREG]
    d = pre - target.astype(np.float64)
    ad = np.abs(d)
    huber = np.where(ad < 1.0, 0.5 * d * d, ad - 0.5)
    l_bk = huber.mean(axis=2)  # [B,K]

    pos_mask = mask.astype(bool)
    factor = np.arctan(l_bk) * (2.0 / np.pi)
    vals = np.where(pos_mask, factor, 1.0)  # [B,K]

    # loss0: smooth-L1 of the last positive in flat (b,k) order
    flat_m = pos_mask.reshape(-1)
    nz = np.nonzero(flat_m)[0]
    loss0 = float(l_bk.reshape(-1)[nz[-1]]) if nz.size else 0.0

    # ---- host: multiplicative scatter + focal corrections ------------------
    b_idx = np.broadcast_to(np.arange(b)[:, None], (b, k)).reshape(-1)
    ch = inde[..., 0].reshape(-1)
    yy = inde[..., 1].reshape(-1)
    xx = inde[..., 2].reshape(-1)
    u = ((b_idx * NCLS + ch) * H + yy) * W + xx  # flat positions into pred
    uu, invmap = np.unique(u, return_inverse=True)
    prod = np.ones(uu.size, dtype=np.float64)
    np.multiply.at(prod, invmap, vals.reshape(-1))

    p_old = pred.reshape(-1)[uu].astype(np.float64)
    p_new = p_old * prod
    gtv_u = gt.reshape(-1)[uu].astype(np.float64)
    g_u = GAMMAS[(uu // (H * W)) % NCLS]
    w_u = g_u * 0.5
    delta = float(
        np.sum(w_u * (_focal_terms(p_new, gtv_u, g_u) - _focal_terms(p_old, gtv_u, g_u)))
    )

    # ---- host: positives (gt == 1.0) — vanishing probability path ----------
    num_pos = 0
    pos_total = 0.0
    if float(gt.max()) >= 1.0:
        pm = gt == np.float32(1.0)
        num_pos = int(pm.sum())
        if num_pos:
            pw = np.where(pm)
            pvals = pred[pw].astype(np.float64)
            gpos = GAMMAS[pw[1]]
            pos_total = float(
                np.sum(gpos * 0.5 * np.log(pvals + EPS) * np.power(1.0 - pvals, gpos))
            )

    loss = loss0 - (S + pos_total + delta)
    if num_pos > 0:
        loss = loss / num_pos
    return np.asarray(np.float32(loss))



# revision 49
# speedup vs baseline: 1.0033x; 1.0033x over previous
"""EqualizedFocalLoss kernel for 8 Trainium2 NeuronCores.

Strategy
--------
The loss is dominated by the focal reduction over pred/gt ([32,15,256,256]
f32 each, ~125.8 MB per tensor).  That part is memory-bound and runs on
device, data-parallel over batch (4 batches per core):

    device S = sum_c (gamma_c/2) * sum_{b,h,w} ln(1-p+eps) * p^gamma_c * (1-gt)^4

computed at the *unmodified* pred.  Everything index-sized — the [B,K]
gather + smooth-L1, the multiplicative scatter (which touches at most
B*K = 16000 positions), the correction of the focal sum at those
positions, loss0, and num_pos handling — is exact fp64 host math.

Per core the device program streams 15 channel-tiles of [128, 4*512]
(fp32 in HBM, fp16 intermediates — fp16 keeps DVE's 2x mode and avoids the
bf16 correlated-rounding bias of the (1-gt)^4 chain):
  ACT   : lq = Ln(1-p); pg = Exp(g*Ln(p+eps) + ln(g/2)), or Square for
          gamma in {2, 3} (all three live in one activation-table set)
  GPSIMD: omg = 1 - gt
  DVE   : w2 = omg^2, nw = w2^2, t1 = lq*pg, t2 = t1*nw   (fp16, 2x mode)
  PE    : ones[128,1].T @ t2-chunks accumulated into one PSUM [1,512] row
Exp-path channels run first and the cheap Square-path channels last, with
the final two channels split into per-plane chunks, so the post-DMA tail is
short; the kernel is DMA-bound at ~87us of 100us predicted total.

Tail refinement over the first pass: the final channel's last-plane
p^2 runs on ACT Square and its (1-gt)^4 squaring on Pool — both idle by
then — instead of queueing behind Pool's and DVE's saturated in-order
tail streams, and only the Exp-path ln(g/2) biases are memset in the
preamble, so the first DMA issues ~0.3us earlier.  (DVE's
tensor_tensor_reduce would shorten the closing chain further but
faults on this hardware, so the reduction stays on PE.)
"""

import math

import numpy as np

B, NCLS, H, W = 32, 15, 256, 256
K, CREG = 500, 2
N_CORES = 8
BPC = B // N_CORES  # batches per core
HW = H * W
P = 128
F = HW // P  # 512
F2 = F // 2  # 256
FREE = BPC * F  # 2048
EPS = 1e-12

GAMMAS = np.array(
    [2.7, 2.1, 2.4, 2.0, 3.0, 2.9, 3.0, 2.5, 2.1, 2.6, 2.0, 2.1, 2.7, 2.4, 2.2],
    dtype=np.float64,
)

_CACHE = {}


def _patch_act_tables(bacc, mybir):
    """Force Bacc's table-load chooser to use natural_log_exp_and_others for
    Ln/Exp/Square so the kernel needs exactly one ACT_TABLE_LOAD instead of
    thrashing between per-function sets.  Only set *membership* is edited —
    dict order (the act_func_set_id mapping) is preserved."""
    if getattr(bacc, "_efl_act_tables_patched", False):
        return
    orig = bacc.get_activation_tables
    ACT = mybir.ActivationFunctionType
    targets = {ACT.Ln, ACT.Exp, ACT.Square}
    keep = "natural_log_exp_and_others"

    def patched(arch):
        tabs = {k: set(v) for k, v in orig(arch).items()}
        if keep in tabs:
            prot = tabs[keep] & targets
            for name, s in tabs.items():
                if name != keep:
                    s -= prot
        return tabs

    bacc.get_activation_tables = patched
    bacc._efl_act_tables_patched = True


def _build_bass():
    import concourse.tile as tile
    from concourse import bacc, mybir

    _patch_act_tables(bacc, mybir)
    nc = bacc.Bacc()
    pred = nc.dram_tensor(
        "pred", [BPC, NCLS, HW], mybir.dt.float32, kind="ExternalInput"
    )
    gt = nc.dram_tensor("gt", [BPC, NCLS, HW], mybir.dt.float32, kind="ExternalInput")
    out1 = nc.dram_tensor("out1", [1, F], mybir.dt.float32, kind="ExternalOutput")

    fdt = mybir.dt.float32
    bdt = mybir.dt.float16
    ALU = mybir.AluOpType
    ACT = mybir.ActivationFunctionType

    # Register activation-bias constants the same way Bass registers its
    # built-in const APs: memset before an all-engine barrier, so later reads
    # need no semaphore waits (the AC instruction has very few sync slots).
    _eng = [nc.gpsimd, nc.vector]

    def register_const(value):
        key = (fdt, value)
        if key in nc.const_aps.aps:
            return
        t = nc.alloc_sbuf_tensor(f"kconst-{len(nc.const_aps.aps)}", [P, 1], fdt)
        _eng[len(nc.const_aps.aps) % len(_eng)].memset(t.ap(), value)
        nc.const_aps.aps[key] = t.ap()

    register_const(EPS)
    for _g in sorted(set(GAMMAS.tolist())):
        if _g not in (2.0, 3.0):  # only Exp-path channels read ln(g/2)
            register_const(math.log(_g / 2.0))
    nc.all_engine_barrier()

    with tile.TileContext(nc) as tc:
        with (
            tc.tile_pool(name="io", bufs=4) as io_pool,
            tc.tile_pool(name="mid", bufs=3) as mid_pool,
            tc.tile_pool(name="fix", bufs=1) as fix_pool,
            tc.tile_pool(name="psum", bufs=1, space="PSUM") as psum_pool,
        ):
            ones = fix_pool.tile([P, 1], bdt)
            nc.vector.memset(ones, 1.0)
            # gamma/2 = 1.5 for the gamma==3 channels, exact in fp16
            ones15 = fix_pool.tile([P, 1], bdt, tag="ones15")
            nc.vector.memset(ones15, 1.5)
            out_t = fix_pool.tile([1, F], fdt, tag="outt")
            psum_f = psum_pool.tile([1, F], mybir.dt.float32)

            # Warm the Ln/Exp activation tables on dependency-free dummy ops so
            # walrus attaches ACT_TABLE_LOAD to an instruction with no waits.
            warm = fix_pool.tile([P, 1], fdt, tag="warm")
            const1 = nc.const_aps.tensor(1.0, (P, 1))
            nc.scalar.activation(out=warm, in_=const1, func=ACT.Ln, bias=1.0)
            nc.scalar.activation(out=warm, in_=const1, func=ACT.Exp, bias=0.0)

            pred_r = pred[:].rearrange("b c (p f) -> c p b f", p=P)
            gt_r = gt[:].rearrange("b c (p f) -> c p b f", p=P)

            # Process the Exp-path channels (3 ACT passes, slower than the
            # 5.83us/channel DMA rate) first and the cheap Square-path
            # channels (2 ACT passes) last, so ACT drains its backlog before
            # the final tile and the post-DMA tail stays short.
            order = (
                [c for c in range(NCLS) if float(GAMMAS[c]) == 3.0]
                + [c for c in range(NCLS) if float(GAMMAS[c]) not in (2.0, 3.0)]
                + [c for c in range(NCLS) if float(GAMMAS[c]) == 2.0]
            )
            for ci, c in enumerate(order):
                g = float(GAMMAS[c])
                last = ci == NCLS - 1
                tailish = ci >= NCLS - 2
                p_t = io_pool.tile([P, BPC, F], fdt, tag="p")
                g_t = io_pool.tile([P, BPC, F], fdt, tag="g")
                if tailish and not last:
                    # Second-to-last channel: gt lands first as one transfer
                    # (its full-tile omg/w2/nw run early, off the tail);
                    # pred is chunked for the pipelined pred-side below.
                    nc.sync.dma_start(out=g_t, in_=gt_r[c])
                    for j in range(BPC):
                        nc.sync.dma_start(out=p_t[:, j], in_=pred_r[c][:, j])
                elif last:
                    # Final channel: interleave gt/pred per plane so the
                    # chunked chain starts as soon as the first planes land.
                    for j in range(BPC):
                        nc.sync.dma_start(out=g_t[:, j], in_=gt_r[c][:, j])
                        nc.sync.dma_start(out=p_t[:, j], in_=pred_r[c][:, j])
                else:
                    nc.sync.dma_start(out=p_t, in_=pred_r[c])
                    nc.sync.dma_start(out=g_t, in_=gt_r[c])
                p2 = p_t.rearrange("p b f -> p (b f)")
                g2 = g_t.rearrange("p b f -> p (b f)")

                omg = mid_pool.tile([P, FREE], bdt, tag="omg")
                w2 = mid_pool.tile([P, FREE], bdt, tag="w2")
                nw = mid_pool.tile([P, FREE], bdt, tag="nw")
                lq = mid_pool.tile([P, FREE], bdt, tag="lq")
                pg = mid_pool.tile([P, FREE], bdt, tag="pg")
                p2sq = mid_pool.tile([P, FREE], bdt, tag="p2sq")
                lp = mid_pool.tile([P, FREE], fdt, tag="lp")
                t1 = mid_pool.tile([P, FREE], bdt, tag="t1")
                t2 = mid_pool.tile([P, FREE], bdt, tag="t2")
                lhsT = ones15 if g == 3.0 else ones

                # Pipeline the final channel in per-plane chunks (last plane
                # halved) so the post-DMA tail is a few small ops instead of
                # full-tile ones.
                chunks = (
                    [slice(j * F, (j + 1) * F) for j in range(BPC)]
                    if tailish
                    else [slice(0, FREE)]
                )
                if tailish and not last:
                    fullsl = slice(0, FREE)
                    nc.gpsimd.tensor_scalar(
                        out=omg[:, fullsl], in0=g2[:, fullsl], scalar1=-1.0,
                        scalar2=1.0, op0=ALU.mult, op1=ALU.add,
                    )
                    nc.vector.tensor_tensor(
                        out=w2[:, fullsl], in0=omg[:, fullsl],
                        in1=omg[:, fullsl], op=ALU.mult,
                    )
                    nc.vector.tensor_tensor(
                        out=nw[:, fullsl], in0=w2[:, fullsl],
                        in1=w2[:, fullsl], op=ALU.mult,
                    )

                for ki, sl in enumerate(chunks):
                    lastchunk = last and ki == len(chunks) - 1
                    if tailish and not last:
                        pass  # gt side computed at full tile above
                    elif last:
                        # Tail channel: w2 = Square(-gt+1) straight from gt
                        # on ACT (drops Pool's omg from the tail chain).
                        # The final chunk's nw runs on Pool (idle by then),
                        # keeping DVE's closing queue minimal.
                        nc.scalar.activation(
                            out=w2[:, sl], in_=g2[:, sl], func=ACT.Square,
                            bias=1.0, scale=-1.0,
                        )
                        nw_eng = nc.gpsimd if lastchunk else nc.vector
                        nw_eng.tensor_tensor(
                            out=nw[:, sl], in0=w2[:, sl], in1=w2[:, sl],
                            op=ALU.mult,
                        )
                    else:
                        # gt side: omg = 1-gt (Pool; ~2.9us/tile vs DVE
                        # 1.13us, but Pool is far under the DMA floor while
                        # DVE is not)
                        nc.gpsimd.tensor_scalar(
                            out=omg[:, sl], in0=g2[:, sl], scalar1=-1.0,
                            scalar2=1.0, op0=ALU.mult, op1=ALU.add,
                        )
                        nc.vector.tensor_tensor(
                            out=w2[:, sl], in0=omg[:, sl], in1=omg[:, sl],
                            op=ALU.mult,
                        )
                        nc.vector.tensor_tensor(
                            out=nw[:, sl], in0=w2[:, sl], in1=w2[:, sl],
                            op=ALU.mult,
                        )

                    nc.scalar.activation(
                        out=lq[:, sl], in_=p2[:, sl], func=ACT.Ln, bias=1.0,
                        scale=-1.0,
                    )
                    if g == 2.0:
                        # (g/2)*p^g == p^2 exactly.  For the final channel's
                        # full planes compute it on idle GPSIMD; its last two
                        # half-chunks use DVE (short critical chain);
                        # otherwise ACT Square (same table set as Ln/Exp).
                        if last and ki == len(chunks) - 1:
                            # Final chunk: p^2 on ACT, which has drained by
                            # now — Pool's queue would deliver it ~1.5us
                            # later and gate the closing chain.
                            nc.scalar.activation(
                                out=pg[:, sl], in_=p2[:, sl], func=ACT.Square
                            )
                        elif last:
                            nc.gpsimd.tensor_tensor(
                                out=pg[:, sl], in0=p2[:, sl], in1=p2[:, sl],
                                op=ALU.mult,
                            )
                        else:
                            nc.scalar.activation(
                                out=pg[:, sl], in_=p2[:, sl], func=ACT.Square
                            )
                    elif g == 3.0:
                        # p^2 on ACT, * p on DVE; the g/2 = 1.5 factor rides
                        # on the matmul's lhsT (ones15).
                        nc.scalar.activation(
                            out=p2sq[:, sl], in_=p2[:, sl], func=ACT.Square
                        )
                        nc.vector.tensor_tensor(
                            out=pg[:, sl], in0=p2sq[:, sl], in1=p2[:, sl],
                            op=ALU.mult,
                        )
                    else:
                        nc.scalar.activation(
                            out=lp[:, sl], in_=p2[:, sl], func=ACT.Ln, bias=EPS,
                            scale=1.0,
                        )
                        nc.scalar.activation(
                            out=pg[:, sl], in_=lp[:, sl], func=ACT.Exp,
                            bias=math.log(g / 2.0), scale=g,
                        )

                    t1_eng = nc.gpsimd if (tailish and not last) else nc.vector
                    t1_eng.tensor_tensor(
                        out=t1[:, sl], in0=lq[:, sl], in1=pg[:, sl], op=ALU.mult
                    )
                    nc.vector.tensor_tensor(
                        out=t2[:, sl], in0=t1[:, sl], in1=nw[:, sl],
                        op=ALU.mult,
                    )
                    t2v = t2[:, sl].rearrange("p (n f) -> p n f", f=F)
                    nsub = (sl.stop - sl.start) // F
                    for j in range(nsub):
                        nc.tensor.matmul(
                            psum_f,
                            lhsT,
                            t2v[:, j],
                            start=(ci == 0 and ki == 0 and j == 0),
                            stop=(
                                last
                                and ki == len(chunks) - 1
                                and j == nsub - 1
                            ),
                        )

            nc.scalar.copy(out=out_t, in_=psum_f)
            nc.sync.dma_start(out=out1[:], in_=out_t)

    nc.finalize()
    return nc


def _device_focal_sums(pred, gt):
    """Run the Bass kernel on 8 cores. Returns per-core partial sums of
    sum_c (g_c/2)*ln(1-p+eps)*p^g_c*(1-gt)^4 over that core's batches."""
    from concourse.bass_utils import run_bass_kernel_spmd

    if "nc" not in _CACHE:
        _CACHE["nc"] = _build_bass()
    nc = _CACHE["nc"]

    in_maps = []
    for i in range(N_CORES):
        sl = slice(i * BPC, (i + 1) * BPC)
        in_maps.append(
            {
                "pred": np.ascontiguousarray(pred[sl]).reshape(BPC, NCLS, HW),
                "gt": np.ascontiguousarray(gt[sl]).reshape(BPC, NCLS, HW),
            }
        )
    last_exc = None
    for _attempt in range(3):
        try:
            res = run_bass_kernel_spmd(nc, in_maps, core_ids=list(range(N_CORES)))
            return [
                float(np.sum(r["out1"].astype(np.float64))) for r in res.results
            ]
        except Exception as e:  # transient NRT_EXEC_UNIT_UNRECOVERABLE on axon
            last_exc = e
            import time as _time

            _time.sleep(5.0)
    raise last_exc


def _host_focal_sum(pred, gt):
    """fp64 host fallback for the bulk focal sum (used only when pred has
    values >= 1.0, where the device's eps-free ln(1-p) would diverge from
    the reference)."""
    S = 0.0
    for c in range(NCLS):
        p = pred[:, c].astype(np.float64)
        gv = gt[:, c].astype(np.float64)
        S += (
            GAMMAS[c]
            * 0.5
            * float(
                np.sum(
                    np.log1p(EPS - p)
                    * np.power(p, GAMMAS[c])
                    * np.power(1.0 - gv, 4)
                )
            )
        )
    return S


def _focal_terms(p, gtv, g):
    """Per-element focal contribution (reference formulas, fp64).
    neg part + pos part; pos only where gt == 1."""
    neg = np.log1p(EPS - p) * np.power(p, g) * np.power(1.0 - gtv, 4)
    pos_mask = gtv == 1.0
    pos = np.where(
        pos_mask, np.log(p + EPS) * np.power(1.0 - p, g), 0.0
    )
    return neg + pos


def kernel(**inputs):
    pred = np.asarray(inputs["pred"], dtype=np.float32)
    gt = np.asarray(inputs["gt"], dtype=np.float32)
    output = np.asarray(inputs["output"], dtype=np.float32)
    mask = np.asarray(inputs["mask"])
    ind = np.asarray(inputs["ind"]).astype(np.int64)
    target = np.asarray(inputs["target"], dtype=np.float32)
    inde = np.asarray(inputs["inde"]).astype(np.int64)

    b, c_out = output.shape[0], output.shape[1]
    k = ind.shape[1]

    # ---- device: bulk focal reduction at unmodified pred -------------------
    if float(pred.max()) >= 1.0:
        # Out-of-distribution input (spec: uniform [0,1)); the device path
        # computes ln(1-p) without eps, which only differs when p >= 1.
        S = _host_focal_sum(pred, gt)
    else:
        S = float(sum(_device_focal_sums(pred, gt)))

    # ---- host: gather + smooth-L1 + vals (fp64) ----------------------------
    o2 = output.reshape(b, c_out, -1).astype(np.float64)
    pre = np.stack(
        [np.take_along_axis(o2[:, c, :], ind, axis=1) for c in range(c_out)], axis=2
    )  # [B,K,CREG]
    d = pre - target.astype(np.float64)
    ad = np.abs(d)
    huber = np.where(ad < 1.0, 0.5 * d * d, ad - 0.5)
    l_bk = huber.mean(axis=2)  # [B,K]

    pos_mask = mask.astype(bool)
    factor = np.arctan(l_bk) * (2.0 / np.pi)
    vals = np.where(pos_mask, factor, 1.0)  # [B,K]

    # loss0: smooth-L1 of the last positive in flat (b,k) order
    flat_m = pos_mask.reshape(-1)
    nz = np.nonzero(flat_m)[0]
    loss0 = float(l_bk.reshape(-1)[nz[-1]]) if nz.size else 0.0

    # ---- host: multiplicative scatter + focal corrections ------------------
    b_idx = np.broadcast_to(np.arange(b)[:, None], (b, k)).reshape(-1)
    ch = inde[..., 0].reshape(-1)
    yy = inde[..., 1].reshape(-1)
    xx = inde[..., 2].reshape(-1)
    u = ((b_idx * NCLS + ch) * H + yy) * W + xx  # flat positions into pred
    uu, invmap = np.unique(u, return_inverse=True)
    prod = np.ones(uu.size, dtype=np.float64)
    np.multiply.at(prod, invmap, vals.reshape(-1))

    p_old = pred.reshape(-1)[uu].astype(np.float64)
    p_new = p_old * prod
    gtv_u = gt.reshape(-1)[uu].astype(np.float64)
    g_u = GAMMAS[(uu // (H * W)) % NCLS]
    w_u = g_u * 0.5
    delta = float(
        np.sum(w_u * (_focal_terms(p_new, gtv_u, g_u) - _focal_terms(p_old, gtv_u, g_u)))
    )

    # ---- host: positives (gt == 1.0) — vanishing probability path ----------
    num_pos = 0
    pos_total = 0.0
    if float(gt.max()) >= 1.0:
        pm = gt == np.float32(1.0)
        num_pos = int(pm.sum())
        if num_pos:
            pw = np.where(pm)
            pvals = pred[pw].astype(np.float64)
            gpos = GAMMAS[pw[1]]
            pos_total = float(
                np.sum(gpos * 0.5 * np.log(pvals + EPS) * np.power(1.0 - pvals, gpos))
            )

    loss = loss0 - (S + pos_total + delta)
    if num_pos > 0:
        loss = loss / num_pos
    return np.asarray(np.float32(loss))


# revision 54
# speedup vs baseline: 1.0056x; 1.0023x over previous
"""EqualizedFocalLoss kernel for 8 Trainium2 NeuronCores.

Strategy
--------
The loss is dominated by the focal reduction over pred/gt ([32,15,256,256]
f32 each, ~125.8 MB per tensor).  That part is memory-bound and runs on
device, data-parallel over batch (4 batches per core):

    device S = sum_c (gamma_c/2) * sum_{b,h,w} ln(1-p+eps) * p^gamma_c * (1-gt)^4

computed at the *unmodified* pred.  Everything index-sized — the [B,K]
gather + smooth-L1, the multiplicative scatter (which touches at most
B*K = 16000 positions), the correction of the focal sum at those
positions, loss0, and num_pos handling — is exact fp64 host math.

Per core the device program streams 15 channel-tiles of [128, 4*512]
(fp32 in HBM, fp16 intermediates — fp16 keeps DVE's 2x mode and avoids the
bf16 correlated-rounding bias of the (1-gt)^4 chain):
  ACT   : lq = Ln(1-p); pg = Exp(g*Ln(p+eps) + ln(g/2)), or Square for
          gamma in {2, 3} (all three live in one activation-table set)
  GPSIMD: omg = 1 - gt
  DVE   : w2 = omg^2, nw = w2^2, t1 = lq*pg, t2 = t1*nw   (fp16, 2x mode)
  PE    : ones[128,1].T @ t2-chunks accumulated into one PSUM [1,512] row
Exp-path channels run first and the cheap Square-path channels last, with
the final two channels split into per-plane chunks, so the post-DMA tail is
short; the kernel is DMA-bound at ~87us of 100us predicted total.

Tail refinement over the first pass: the final channel's last-plane
p^2 runs on ACT Square and its (1-gt)^4 squaring on Pool — both idle by
then — instead of queueing behind Pool's and DVE's saturated in-order
tail streams, and only the Exp-path ln(g/2) biases are memset in the
preamble, so the first DMA issues ~0.3us earlier.  (DVE's
tensor_tensor_reduce would shorten the closing chain further but
faults on this hardware, so the reduction stays on PE.)
"""

import math

import numpy as np

B, NCLS, H, W = 32, 15, 256, 256
K, CREG = 500, 2
N_CORES = 8
BPC = B // N_CORES  # batches per core
HW = H * W
P = 128
F = HW // P  # 512
F2 = F // 2  # 256
FREE = BPC * F  # 2048
EPS = 1e-12

GAMMAS = np.array(
    [2.7, 2.1, 2.4, 2.0, 3.0, 2.9, 3.0, 2.5, 2.1, 2.6, 2.0, 2.1, 2.7, 2.4, 2.2],
    dtype=np.float64,
)

_CACHE = {}


def _patch_act_tables(bacc, mybir):
    """Force Bacc's table-load chooser to use natural_log_exp_and_others for
    Ln/Exp/Square so the kernel needs exactly one ACT_TABLE_LOAD instead of
    thrashing between per-function sets.  Only set *membership* is edited —
    dict order (the act_func_set_id mapping) is preserved."""
    if getattr(bacc, "_efl_act_tables_patched", False):
        return
    orig = bacc.get_activation_tables
    ACT = mybir.ActivationFunctionType
    targets = {ACT.Ln, ACT.Exp, ACT.Square}
    keep = "natural_log_exp_and_others"

    def patched(arch):
        tabs = {k: set(v) for k, v in orig(arch).items()}
        if keep in tabs:
            prot = tabs[keep] & targets
            for name, s in tabs.items():
                if name != keep:
                    s -= prot
        return tabs

    bacc.get_activation_tables = patched
    bacc._efl_act_tables_patched = True


def _build_bass():
    import concourse.tile as tile
    from concourse import bacc, mybir

    _patch_act_tables(bacc, mybir)
    nc = bacc.Bacc()
    pred = nc.dram_tensor(
        "pred", [BPC, NCLS, HW], mybir.dt.float32, kind="ExternalInput"
    )
    gt = nc.dram_tensor("gt", [BPC, NCLS, HW], mybir.dt.float32, kind="ExternalInput")
    out1 = nc.dram_tensor("out1", [1, F], mybir.dt.float32, kind="ExternalOutput")

    fdt = mybir.dt.float32
    bdt = mybir.dt.float16
    ALU = mybir.AluOpType
    ACT = mybir.ActivationFunctionType

    # Register activation-bias constants the same way Bass registers its
    # built-in const APs: memset before an all-engine barrier, so later reads
    # need no semaphore waits (the AC instruction has very few sync slots).
    _eng = [nc.gpsimd, nc.vector]

    def register_const(value):
        key = (fdt, value)
        if key in nc.const_aps.aps:
            return
        t = nc.alloc_sbuf_tensor(f"kconst-{len(nc.const_aps.aps)}", [P, 1], fdt)
        _eng[len(nc.const_aps.aps) % len(_eng)].memset(t.ap(), value)
        nc.const_aps.aps[key] = t.ap()

    register_const(EPS)
    for _g in sorted(set(GAMMAS.tolist())):
        if _g not in (2.0, 3.0):  # only Exp-path channels read ln(g/2)
            register_const(math.log(_g / 2.0))
    nc.all_engine_barrier()

    with tile.TileContext(nc) as tc:
        with (
            tc.tile_pool(name="io", bufs=4) as io_pool,
            tc.tile_pool(name="mid", bufs=3) as mid_pool,
            tc.tile_pool(name="fix", bufs=1) as fix_pool,
            tc.tile_pool(name="psum", bufs=1, space="PSUM") as psum_pool,
        ):
            ones = fix_pool.tile([P, 1], bdt)
            nc.vector.memset(ones, 1.0)
            # gamma/2 = 1.5 for the gamma==3 channels, exact in fp16
            ones15 = fix_pool.tile([P, 1], bdt, tag="ones15")
            nc.vector.memset(ones15, 1.5)
            out_t = fix_pool.tile([1, F], fdt, tag="outt")
            psum_f = psum_pool.tile([1, F], mybir.dt.float32)

            # Warm the Ln/Exp activation tables on dependency-free dummy ops so
            # walrus attaches ACT_TABLE_LOAD to an instruction with no waits.
            warm = fix_pool.tile([P, 1], fdt, tag="warm")
            const1 = nc.const_aps.tensor(1.0, (P, 1))
            nc.scalar.activation(out=warm, in_=const1, func=ACT.Ln, bias=1.0)
            nc.scalar.activation(out=warm, in_=const1, func=ACT.Exp, bias=0.0)

            pred_r = pred[:].rearrange("b c (p f) -> c p b f", p=P)
            gt_r = gt[:].rearrange("b c (p f) -> c p b f", p=P)

            # Process the Exp-path channels (3 ACT passes, slower than the
            # 5.83us/channel DMA rate) first and the cheap Square-path
            # channels (2 ACT passes) last, so ACT drains its backlog before
            # the final tile and the post-DMA tail stays short.
            order = (
                [c for c in range(NCLS) if float(GAMMAS[c]) == 3.0]
                + [c for c in range(NCLS) if float(GAMMAS[c]) not in (2.0, 3.0)]
                + [c for c in range(NCLS) if float(GAMMAS[c]) == 2.0]
            )
            for ci, c in enumerate(order):
                g = float(GAMMAS[c])
                last = ci == NCLS - 1
                tailish = ci >= NCLS - 2
                p_t = io_pool.tile([P, BPC, F], fdt, tag="p")
                g_t = io_pool.tile([P, BPC, F], fdt, tag="g")
                if tailish and not last:
                    # Second-to-last channel: gt lands first as one transfer
                    # (its full-tile omg/w2/nw run early, off the tail);
                    # pred is chunked for the pipelined pred-side below.
                    nc.sync.dma_start(out=g_t, in_=gt_r[c])
                    for j in range(BPC):
                        nc.sync.dma_start(out=p_t[:, j], in_=pred_r[c][:, j])
                elif last:
                    # Final channel: interleave gt/pred per plane so the
                    # chunked chain starts as soon as the first planes land.
                    for j in range(BPC):
                        nc.sync.dma_start(out=g_t[:, j], in_=gt_r[c][:, j])
                        nc.sync.dma_start(out=p_t[:, j], in_=pred_r[c][:, j])
                else:
                    nc.sync.dma_start(out=p_t, in_=pred_r[c])
                    nc.sync.dma_start(out=g_t, in_=gt_r[c])
                p2 = p_t.rearrange("p b f -> p (b f)")
                g2 = g_t.rearrange("p b f -> p (b f)")

                omg = mid_pool.tile([P, FREE], bdt, tag="omg")
                w2 = mid_pool.tile([P, FREE], bdt, tag="w2")
                nw = mid_pool.tile([P, FREE], bdt, tag="nw")
                lq = mid_pool.tile([P, FREE], bdt, tag="lq")
                pg = mid_pool.tile([P, FREE], bdt, tag="pg")
                p2sq = mid_pool.tile([P, FREE], bdt, tag="p2sq")
                lp = mid_pool.tile([P, FREE], fdt, tag="lp")
                t1 = mid_pool.tile([P, FREE], bdt, tag="t1")
                t2 = mid_pool.tile([P, FREE], bdt, tag="t2")
                lhsT = ones15 if g == 3.0 else ones

                # Pipeline the final channel in per-plane chunks (last plane
                # halved) so the post-DMA tail is a few small ops instead of
                # full-tile ones.
                chunks = (
                    [slice(j * F, (j + 1) * F) for j in range(BPC)]
                    if tailish
                    else [slice(0, FREE)]
                )
                if tailish and not last:
                    fullsl = slice(0, FREE)
                    nc.gpsimd.tensor_scalar(
                        out=omg[:, fullsl], in0=g2[:, fullsl], scalar1=-1.0,
                        scalar2=1.0, op0=ALU.mult, op1=ALU.add,
                    )
                    nc.vector.tensor_tensor(
                        out=w2[:, fullsl], in0=omg[:, fullsl],
                        in1=omg[:, fullsl], op=ALU.mult,
                    )
                    nc.vector.tensor_tensor(
                        out=nw[:, fullsl], in0=w2[:, fullsl],
                        in1=w2[:, fullsl], op=ALU.mult,
                    )

                for ki, sl in enumerate(chunks):
                    lastchunk = last and ki == len(chunks) - 1
                    if tailish and not last:
                        pass  # gt side computed at full tile above
                    elif last:
                        # Tail channel: w2 = Square(-gt+1) straight from gt
                        # on ACT (drops Pool's omg from the tail chain).
                        # The final chunk's nw runs on Pool (idle by then),
                        # keeping DVE's closing queue minimal.
                        nc.scalar.activation(
                            out=w2[:, sl], in_=g2[:, sl], func=ACT.Square,
                            bias=1.0, scale=-1.0,
                        )
                        nw_eng = nc.gpsimd if lastchunk else nc.vector
                        nw_eng.tensor_tensor(
                            out=nw[:, sl], in0=w2[:, sl], in1=w2[:, sl],
                            op=ALU.mult,
                        )
                    else:
                        # gt side: omg = 1-gt (Pool; ~2.9us/tile vs DVE
                        # 1.13us, but Pool is far under the DMA floor while
                        # DVE is not)
                        nc.gpsimd.tensor_scalar(
                            out=omg[:, sl], in0=g2[:, sl], scalar1=-1.0,
                            scalar2=1.0, op0=ALU.mult, op1=ALU.add,
                        )
                        nc.vector.tensor_tensor(
                            out=w2[:, sl], in0=omg[:, sl], in1=omg[:, sl],
                            op=ALU.mult,
                        )
                        nc.vector.tensor_tensor(
                            out=nw[:, sl], in0=w2[:, sl], in1=w2[:, sl],
                            op=ALU.mult,
                        )

                    nc.scalar.activation(
                        out=lq[:, sl], in_=p2[:, sl], func=ACT.Ln, bias=1.0,
                        scale=-1.0,
                    )
                    if g == 2.0:
                        # (g/2)*p^g == p^2 exactly.  For the final channel's
                        # full planes compute it on idle GPSIMD; its last two
                        # half-chunks use DVE (short critical chain);
                        # otherwise ACT Square (same table set as Ln/Exp).
                        if last and ki >= 2:
                            # Last two planes: p^2 on ACT, which has drained
                            # by now — Pool's in-order queue would deliver
                            # them ~1.5us later and gate the closing chain.
                            nc.scalar.activation(
                                out=pg[:, sl], in_=p2[:, sl], func=ACT.Square
                            )
                        elif last:
                            nc.gpsimd.tensor_tensor(
                                out=pg[:, sl], in0=p2[:, sl], in1=p2[:, sl],
                                op=ALU.mult,
                            )
                        else:
                            nc.scalar.activation(
                                out=pg[:, sl], in_=p2[:, sl], func=ACT.Square
                            )
                    elif g == 3.0:
                        # p^2 on ACT, * p on DVE; the g/2 = 1.5 factor rides
                        # on the matmul's lhsT (ones15).
                        nc.scalar.activation(
                            out=p2sq[:, sl], in_=p2[:, sl], func=ACT.Square
                        )
                        nc.vector.tensor_tensor(
                            out=pg[:, sl], in0=p2sq[:, sl], in1=p2[:, sl],
                            op=ALU.mult,
                        )
                    else:
                        nc.scalar.activation(
                            out=lp[:, sl], in_=p2[:, sl], func=ACT.Ln, bias=EPS,
                            scale=1.0,
                        )
                        nc.scalar.activation(
                            out=pg[:, sl], in_=lp[:, sl], func=ACT.Exp,
                            bias=math.log(g / 2.0), scale=g,
                        )

                    t1_eng = (
                        nc.gpsimd
                        if (tailish and not last) or (last and ki == 2)
                        else nc.vector
                    )
                    t1_eng.tensor_tensor(
                        out=t1[:, sl], in0=lq[:, sl], in1=pg[:, sl], op=ALU.mult
                    )
                    nc.vector.tensor_tensor(
                        out=t2[:, sl], in0=t1[:, sl], in1=nw[:, sl],
                        op=ALU.mult,
                    )
                    t2v = t2[:, sl].rearrange("p (n f) -> p n f", f=F)
                    nsub = (sl.stop - sl.start) // F
                    for j in range(nsub):
                        nc.tensor.matmul(
                            psum_f,
                            lhsT,
                            t2v[:, j],
                            start=(ci == 0 and ki == 0 and j == 0),
                            stop=(
                                last
                                and ki == len(chunks) - 1
                                and j == nsub - 1
                            ),
                        )

            nc.scalar.copy(out=out_t, in_=psum_f)
            nc.sync.dma_start(out=out1[:], in_=out_t)

    nc.finalize()
    return nc


def _device_focal_sums(pred, gt):
    """Run the Bass kernel on 8 cores. Returns per-core partial sums of
    sum_c (g_c/2)*ln(1-p+eps)*p^g_c*(1-gt)^4 over that core's batches."""
    from concourse.bass_utils import run_bass_kernel_spmd

    if "nc" not in _CACHE:
        _CACHE["nc"] = _build_bass()
    nc = _CACHE["nc"]

    in_maps = []
    for i in range(N_CORES):
        sl = slice(i * BPC, (i + 1) * BPC)
        in_maps.append(
            {
                "pred": np.ascontiguousarray(pred[sl]).reshape(BPC, NCLS, HW),
                "gt": np.ascontiguousarray(gt[sl]).reshape(BPC, NCLS, HW),
            }
        )
    last_exc = None
    for _attempt in range(3):
        try:
            res = run_bass_kernel_spmd(nc, in_maps, core_ids=list(range(N_CORES)))
            return [
                float(np.sum(r["out1"].astype(np.float64))) for r in res.results
            ]
        except Exception as e:  # transient NRT_EXEC_UNIT_UNRECOVERABLE on axon
            last_exc = e
            import time as _time

            _time.sleep(5.0)
    raise last_exc


def _host_focal_sum(pred, gt):
    """fp64 host fallback for the bulk focal sum (used only when pred has
    values >= 1.0, where the device's eps-free ln(1-p) would diverge from
    the reference)."""
    S = 0.0
    for c in range(NCLS):
        p = pred[:, c].astype(np.float64)
        gv = gt[:, c].astype(np.float64)
        S += (
            GAMMAS[c]
            * 0.5
            * float(
                np.sum(
                    np.log1p(EPS - p)
                    * np.power(p, GAMMAS[c])
                    * np.power(1.0 - gv, 4)
                )
            )
        )
    return S


def _focal_terms(p, gtv, g):
    """Per-element focal contribution (reference formulas, fp64).
    neg part + pos part; pos only where gt == 1."""
    neg = np.log1p(EPS - p) * np.power(p, g) * np.power(1.0 - gtv, 4)
    pos_mask = gtv == 1.0
    pos = np.where(
        pos_mask, np.log(p + EPS) * np.power(1.0 - p, g), 0.0
    )
    return neg + pos


def kernel(**inputs):
    pred = np.asarray(inputs["pred"], dtype=np.float32)
    gt = np.asarray(inputs["gt"], dtype=np.float32)
    output = np.asarray(inputs["output"], dtype=np.float32)
    mask = np.asarray(inputs["mask"])
    ind = np.asarray(inputs["ind"]).astype(np.int64)
    target = np.asarray(inputs["target"], dtype=np.float32)
    inde = np.asarray(inputs["inde"]).astype(np.int64)

    b, c_out = output.shape[0], output.shape[1]
    k = ind.shape[1]

    # ---- device: bulk focal reduction at unmodified pred -------------------
    if float(pred.max()) >= 1.0:
        # Out-of-distribution input (spec: uniform [0,1)); the device path
        # computes ln(1-p) without eps, which only differs when p >= 1.
        S = _host_focal_sum(pred, gt)
    else:
        S = float(sum(_device_focal_sums(pred, gt)))

    # ---- host: gather + smooth-L1 + vals (fp64) ----------------------------
    o2 = output.reshape(b, c_out, -1).astype(np.float64)
    pre = np.stack(
        [np.take_along_axis(o2[:, c, :], ind, axis=1) for c in range(c_out)], axis=2
    )  # [B,K,CREG]
    d = pre - target.astype(np.float64)
    ad = np.abs(d)
    huber = np.where(ad < 1.0, 0.5 * d * d, ad - 0.5)
    l_bk = huber.mean(axis=2)  # [B,K]

    pos_mask = mask.astype(bool)
    factor = np.arctan(l_bk) * (2.0 / np.pi)
    vals = np.where(pos_mask, factor, 1.0)  # [B,K]

    # loss0: smooth-L1 of the last positive in flat (b,k) order
    flat_m = pos_mask.reshape(-1)
    nz = np.nonzero(flat_m)[0]
    loss0 = float(l_bk.reshape(-1)[nz[-1]]) if nz.size else 0.0

    # ---- host: multiplicative scatter + focal corrections ------------------
    b_idx = np.broadcast_to(np.arange(b)[:, None], (b, k)).reshape(-1)
    ch = inde[..., 0].reshape(-1)
    yy = inde[..., 1].reshape(-1)
    xx = inde[..., 2].reshape(-1)
    u = ((b_idx * NCLS + ch) * H + yy) * W + xx  # flat positions into pred
    uu, invmap = np.unique(u, return_inverse=True)
    prod = np.ones(uu.size, dtype=np.float64)
    np.multiply.at(prod, invmap, vals.reshape(-1))

    p_old = pred.reshape(-1)[uu].astype(np.float64)
    p_new = p_old * prod
    gtv_u = gt.reshape(-1)[uu].astype(np.float64)
    g_u = GAMMAS[(uu // (H * W)) % NCLS]
    w_u = g_u * 0.5
    delta = float(
        np.sum(w_u * (_focal_terms(p_new, gtv_u, g_u) - _focal_terms(p_old, gtv_u, g_u)))
    )

    # ---- host: positives (gt == 1.0) — vanishing probability path ----------
    num_pos = 0
    pos_total = 0.0
    if float(gt.max()) >= 1.0:
        pm = gt == np.float32(1.0)
        num_pos = int(pm.sum())
        if num_pos:
            pw = np.where(pm)
            pvals = pred[pw].astype(np.float64)
            gpos = GAMMAS[pw[1]]
            pos_total = float(
                np.sum(gpos * 0.5 * np.log(pvals + EPS) * np.power(1.0 - pvals, gpos))
            )

    loss = loss0 - (S + pos_total + delta)
    if num_pos > 0:
        loss = loss / num_pos
    return np.asarray(np.float32(loss))


# revision 59
# speedup vs baseline: 1.0113x; 1.0057x over previous
"""EqualizedFocalLoss kernel for 8 Trainium2 NeuronCores.

Strategy
--------
The loss is dominated by the focal reduction over pred/gt ([32,15,256,256]
f32 each, ~125.8 MB per tensor).  That part is memory-bound and runs on
device, data-parallel over batch (4 batches per core):

    device S = sum_c (gamma_c/2) * sum_{b,h,w} ln(1-p+eps) * p^gamma_c * (1-gt)^4

computed at the *unmodified* pred.  Everything index-sized — the [B,K]
gather + smooth-L1, the multiplicative scatter (which touches at most
B*K = 16000 positions), the correction of the focal sum at those
positions, loss0, and num_pos handling — is exact fp64 host math.

Per core the device program streams 15 channel-tiles of [128, 4*512]
(fp32 in HBM, fp16 intermediates — fp16 keeps DVE's 2x mode and avoids the
bf16 correlated-rounding bias of the (1-gt)^4 chain):
  ACT   : lq = Ln(1-p); pg = Exp(g*Ln(p+eps) + ln(g/2)), or Square for
          gamma in {2, 3} (all three live in one activation-table set)
  GPSIMD: omg = 1 - gt
  DVE   : w2 = omg^2, nw = w2^2, t1 = lq*pg, t2 = t1*nw   (fp16, 2x mode)
  PE    : ones[128,1].T @ t2-chunks accumulated into one PSUM [1,512] row
Exp-path channels run first and the cheap Square-path channels last, with
the final two channels split into per-plane chunks, so the post-DMA tail is
short; the kernel is DMA-bound at ~87us of 100us predicted total.

Tail refinement over the first pass: the final channel's last-plane
p^2 runs on ACT Square and its (1-gt)^4 squaring on Pool — both idle by
then — instead of queueing behind Pool's and DVE's saturated in-order
tail streams, and only the Exp-path ln(g/2) biases are memset in the
preamble, so the first DMA issues ~0.3us earlier.  (DVE's
tensor_tensor_reduce would shorten the closing chain further but
faults on this hardware, so the reduction stays on PE.)
"""

import math

import numpy as np

B, NCLS, H, W = 32, 15, 256, 256
K, CREG = 500, 2
N_CORES = 8
BPC = B // N_CORES  # batches per core
HW = H * W
P = 128
F = HW // P  # 512
F2 = F // 2  # 256
FREE = BPC * F  # 2048
EPS = 1e-12

GAMMAS = np.array(
    [2.7, 2.1, 2.4, 2.0, 3.0, 2.9, 3.0, 2.5, 2.1, 2.6, 2.0, 2.1, 2.7, 2.4, 2.2],
    dtype=np.float64,
)

_CACHE = {}


def _patch_act_tables(bacc, mybir):
    """Force Bacc's table-load chooser to use natural_log_exp_and_others for
    Ln/Exp/Square so the kernel needs exactly one ACT_TABLE_LOAD instead of
    thrashing between per-function sets.  Only set *membership* is edited —
    dict order (the act_func_set_id mapping) is preserved."""
    if getattr(bacc, "_efl_act_tables_patched", False):
        return
    orig = bacc.get_activation_tables
    ACT = mybir.ActivationFunctionType
    targets = {ACT.Ln, ACT.Exp, ACT.Square}
    keep = "natural_log_exp_and_others"

    def patched(arch):
        tabs = {k: set(v) for k, v in orig(arch).items()}
        if keep in tabs:
            prot = tabs[keep] & targets
            for name, s in tabs.items():
                if name != keep:
                    s -= prot
        return tabs

    bacc.get_activation_tables = patched
    bacc._efl_act_tables_patched = True


def _build_bass():
    import concourse.tile as tile
    from concourse import bacc, mybir

    _patch_act_tables(bacc, mybir)
    nc = bacc.Bacc()
    pred = nc.dram_tensor(
        "pred", [BPC, NCLS, HW], mybir.dt.float32, kind="ExternalInput"
    )
    gt = nc.dram_tensor("gt", [BPC, NCLS, HW], mybir.dt.float32, kind="ExternalInput")
    out1 = nc.dram_tensor("out1", [1, F], mybir.dt.float32, kind="ExternalOutput")

    fdt = mybir.dt.float32
    bdt = mybir.dt.float16
    ALU = mybir.AluOpType
    ACT = mybir.ActivationFunctionType

    # Register activation-bias constants the same way Bass registers its
    # built-in const APs: memset before an all-engine barrier, so later reads
    # need no semaphore waits (the AC instruction has very few sync slots).
    _eng = [nc.gpsimd, nc.vector]

    def register_const(value):
        key = (fdt, value)
        if key in nc.const_aps.aps:
            return
        t = nc.alloc_sbuf_tensor(f"kconst-{len(nc.const_aps.aps)}", [P, 1], fdt)
        _eng[len(nc.const_aps.aps) % len(_eng)].memset(t.ap(), value)
        nc.const_aps.aps[key] = t.ap()

    register_const(EPS)
    for _g in sorted(set(GAMMAS.tolist())):
        if _g not in (2.0, 3.0):  # only Exp-path channels read ln(g/2)
            register_const(math.log(_g / 2.0))
    # Barrier only the const writers (Pool, DVE) against the reader (ACT):
    # SP stays out, so the first input DMA issues ~1.3us earlier instead of
    # waiting for the preamble to drain.
    nc.multi_engine_barrier(
        [
            mybir.EngineType.Pool,
            mybir.EngineType.DVE,
            mybir.EngineType.Activation,
        ]
    )

    with tile.TileContext(nc) as tc:
        with (
            tc.tile_pool(name="io", bufs=4) as io_pool,
            tc.tile_pool(name="mid", bufs=3) as mid_pool,
            tc.tile_pool(name="fix", bufs=1) as fix_pool,
            tc.tile_pool(name="psum", bufs=1, space="PSUM") as psum_pool,
        ):
            ones = fix_pool.tile([P, 1], bdt)
            nc.vector.memset(ones, 1.0)
            # gamma/2 = 1.5 for the gamma==3 channels, exact in fp16
            ones15 = fix_pool.tile([P, 1], bdt, tag="ones15")
            nc.vector.memset(ones15, 1.5)
            out_t = fix_pool.tile([1, F], fdt, tag="outt")
            psum_f = psum_pool.tile([1, F], mybir.dt.float32)

            # Warm the Ln/Exp activation tables on dependency-free dummy ops so
            # walrus attaches ACT_TABLE_LOAD to an instruction with no waits.
            warm = fix_pool.tile([P, 1], fdt, tag="warm")
            const1 = nc.const_aps.tensor(1.0, (P, 1))
            nc.scalar.activation(out=warm, in_=const1, func=ACT.Ln, bias=1.0)
            nc.scalar.activation(out=warm, in_=const1, func=ACT.Exp, bias=0.0)

            pred_r = pred[:].rearrange("b c (p f) -> c p b f", p=P)
            gt_r = gt[:].rearrange("b c (p f) -> c p b f", p=P)

            # Process the Exp-path channels (3 ACT passes, slower than the
            # 5.83us/channel DMA rate) first and the cheap Square-path
            # channels (2 ACT passes) last, so ACT drains its backlog before
            # the final tile and the post-DMA tail stays short.
            order = (
                [c for c in range(NCLS) if float(GAMMAS[c]) == 3.0]
                + [c for c in range(NCLS) if float(GAMMAS[c]) not in (2.0, 3.0)]
                + [c for c in range(NCLS) if float(GAMMAS[c]) == 2.0]
            )
            for ci, c in enumerate(order):
                g = float(GAMMAS[c])
                last = ci == NCLS - 1
                tailish = ci >= NCLS - 2
                p_t = io_pool.tile([P, BPC, F], fdt, tag="p")
                g_t = io_pool.tile([P, BPC, F], fdt, tag="g")
                if tailish and not last:
                    # Second-to-last channel: gt lands first as one transfer
                    # (its full-tile omg/w2/nw run early, off the tail);
                    # pred is chunked for the pipelined pred-side below.
                    nc.sync.dma_start(out=g_t, in_=gt_r[c])
                    for j in range(BPC):
                        nc.sync.dma_start(out=p_t[:, j], in_=pred_r[c][:, j])
                elif last:
                    # Final channel: interleave gt/pred per plane so the
                    # chunked chain starts as soon as the first planes land.
                    for j in range(BPC):
                        nc.sync.dma_start(out=g_t[:, j], in_=gt_r[c][:, j])
                        nc.sync.dma_start(out=p_t[:, j], in_=pred_r[c][:, j])
                else:
                    nc.sync.dma_start(out=p_t, in_=pred_r[c])
                    nc.sync.dma_start(out=g_t, in_=gt_r[c])
                p2 = p_t.rearrange("p b f -> p (b f)")
                g2 = g_t.rearrange("p b f -> p (b f)")

                omg = mid_pool.tile([P, FREE], bdt, tag="omg")
                w2 = mid_pool.tile([P, FREE], bdt, tag="w2")
                nw = mid_pool.tile([P, FREE], bdt, tag="nw")
                lq = mid_pool.tile([P, FREE], bdt, tag="lq")
                pg = mid_pool.tile([P, FREE], bdt, tag="pg")
                p2sq = mid_pool.tile([P, FREE], bdt, tag="p2sq")
                lp = mid_pool.tile([P, FREE], fdt, tag="lp")
                t1 = mid_pool.tile([P, FREE], bdt, tag="t1")
                t2 = mid_pool.tile([P, FREE], bdt, tag="t2")
                lhsT = ones15 if g == 3.0 else ones

                # Pipeline the final channel in per-plane chunks (last plane
                # halved) so the post-DMA tail is a few small ops instead of
                # full-tile ones.
                chunks = (
                    [slice(j * F, (j + 1) * F) for j in range(BPC)]
                    if tailish
                    else [slice(0, FREE)]
                )
                if tailish and not last:
                    fullsl = slice(0, FREE)
                    nc.gpsimd.tensor_scalar(
                        out=omg[:, fullsl], in0=g2[:, fullsl], scalar1=-1.0,
                        scalar2=1.0, op0=ALU.mult, op1=ALU.add,
                    )
                    nc.vector.tensor_tensor(
                        out=w2[:, fullsl], in0=omg[:, fullsl],
                        in1=omg[:, fullsl], op=ALU.mult,
                    )
                    nc.vector.tensor_tensor(
                        out=nw[:, fullsl], in0=w2[:, fullsl],
                        in1=w2[:, fullsl], op=ALU.mult,
                    )

                for ki, sl in enumerate(chunks):
                    lastchunk = last and ki == len(chunks) - 1
                    if tailish and not last:
                        pass  # gt side computed at full tile above
                    elif last:
                        # Tail channel: w2 = Square(-gt+1) straight from gt
                        # on ACT (drops Pool's omg from the tail chain).
                        # The final chunk's nw runs on Pool (idle by then),
                        # keeping DVE's closing queue minimal.
                        nc.scalar.activation(
                            out=w2[:, sl], in_=g2[:, sl], func=ACT.Square,
                            bias=1.0, scale=-1.0,
                        )
                        nw_eng = nc.gpsimd if lastchunk else nc.vector
                        nw_eng.tensor_tensor(
                            out=nw[:, sl], in0=w2[:, sl], in1=w2[:, sl],
                            op=ALU.mult,
                        )
                    else:
                        # gt side: omg = 1-gt (Pool; ~2.9us/tile vs DVE
                        # 1.13us, but Pool is far under the DMA floor while
                        # DVE is not)
                        nc.gpsimd.tensor_scalar(
                            out=omg[:, sl], in0=g2[:, sl], scalar1=-1.0,
                            scalar2=1.0, op0=ALU.mult, op1=ALU.add,
                        )
                        nc.vector.tensor_tensor(
                            out=w2[:, sl], in0=omg[:, sl], in1=omg[:, sl],
                            op=ALU.mult,
                        )
                        nc.vector.tensor_tensor(
                            out=nw[:, sl], in0=w2[:, sl], in1=w2[:, sl],
                            op=ALU.mult,
                        )

                    nc.scalar.activation(
                        out=lq[:, sl], in_=p2[:, sl], func=ACT.Ln, bias=1.0,
                        scale=-1.0,
                    )
                    if g == 2.0:
                        # (g/2)*p^g == p^2 exactly.  For the final channel's
                        # full planes compute it on idle GPSIMD; its last two
                        # half-chunks use DVE (short critical chain);
                        # otherwise ACT Square (same table set as Ln/Exp).
                        if last and ki >= 2:
                            # Last two planes: p^2 on ACT, which has drained
                            # by now — Pool's in-order queue would deliver
                            # them ~1.5us later and gate the closing chain.
                            nc.scalar.activation(
                                out=pg[:, sl], in_=p2[:, sl], func=ACT.Square
                            )
                        elif last:
                            nc.gpsimd.tensor_tensor(
                                out=pg[:, sl], in0=p2[:, sl], in1=p2[:, sl],
                                op=ALU.mult,
                            )
                        else:
                            nc.scalar.activation(
                                out=pg[:, sl], in_=p2[:, sl], func=ACT.Square
                            )
                    elif g == 3.0:
                        # p^2 on ACT, * p on DVE; the g/2 = 1.5 factor rides
                        # on the matmul's lhsT (ones15).
                        nc.scalar.activation(
                            out=p2sq[:, sl], in_=p2[:, sl], func=ACT.Square
                        )
                        nc.vector.tensor_tensor(
                            out=pg[:, sl], in0=p2sq[:, sl], in1=p2[:, sl],
                            op=ALU.mult,
                        )
                    else:
                        nc.scalar.activation(
                            out=lp[:, sl], in_=p2[:, sl], func=ACT.Ln, bias=EPS,
                            scale=1.0,
                        )
                        nc.scalar.activation(
                            out=pg[:, sl], in_=lp[:, sl], func=ACT.Exp,
                            bias=math.log(g / 2.0), scale=g,
                        )

                    t1_eng = (
                        nc.gpsimd
                        if (tailish and not last) or (last and ki == 2)
                        else nc.vector
                    )
                    t1_eng.tensor_tensor(
                        out=t1[:, sl], in0=lq[:, sl], in1=pg[:, sl], op=ALU.mult
                    )
                    nc.vector.tensor_tensor(
                        out=t2[:, sl], in0=t1[:, sl], in1=nw[:, sl],
                        op=ALU.mult,
                    )
                    t2v = t2[:, sl].rearrange("p (n f) -> p n f", f=F)
                    nsub = (sl.stop - sl.start) // F
                    for j in range(nsub):
                        nc.tensor.matmul(
                            psum_f,
                            lhsT,
                            t2v[:, j],
                            start=(ci == 0 and ki == 0 and j == 0),
                            stop=(
                                last
                                and ki == len(chunks) - 1
                                and j == nsub - 1
                            ),
                        )

            nc.scalar.copy(out=out_t, in_=psum_f)
            nc.sync.dma_start(out=out1[:], in_=out_t)

    nc.finalize()
    return nc


def _device_focal_sums(pred, gt):
    """Run the Bass kernel on 8 cores. Returns per-core partial sums of
    sum_c (g_c/2)*ln(1-p+eps)*p^g_c*(1-gt)^4 over that core's batches."""
    from concourse.bass_utils import run_bass_kernel_spmd

    if "nc" not in _CACHE:
        _CACHE["nc"] = _build_bass()
    nc = _CACHE["nc"]

    in_maps = []
    for i in range(N_CORES):
        sl = slice(i * BPC, (i + 1) * BPC)
        in_maps.append(
            {
                "pred": np.ascontiguousarray(pred[sl]).reshape(BPC, NCLS, HW),
                "gt": np.ascontiguousarray(gt[sl]).reshape(BPC, NCLS, HW),
            }
        )
    last_exc = None
    for _attempt in range(3):
        try:
            res = run_bass_kernel_spmd(nc, in_maps, core_ids=list(range(N_CORES)))
            return [
                float(np.sum(r["out1"].astype(np.float64))) for r in res.results
            ]
        except Exception as e:  # transient NRT_EXEC_UNIT_UNRECOVERABLE on axon
            last_exc = e
            import time as _time

            _time.sleep(5.0)
    raise last_exc


def _host_focal_sum(pred, gt):
    """fp64 host fallback for the bulk focal sum (used only when pred has
    values >= 1.0, where the device's eps-free ln(1-p) would diverge from
    the reference)."""
    S = 0.0
    for c in range(NCLS):
        p = pred[:, c].astype(np.float64)
        gv = gt[:, c].astype(np.float64)
        S += (
            GAMMAS[c]
            * 0.5
            * float(
                np.sum(
                    np.log1p(EPS - p)
                    * np.power(p, GAMMAS[c])
                    * np.power(1.0 - gv, 4)
                )
            )
        )
    return S


def _focal_terms(p, gtv, g):
    """Per-element focal contribution (reference formulas, fp64).
    neg part + pos part; pos only where gt == 1."""
    neg = np.log1p(EPS - p) * np.power(p, g) * np.power(1.0 - gtv, 4)
    pos_mask = gtv == 1.0
    pos = np.where(
        pos_mask, np.log(p + EPS) * np.power(1.0 - p, g), 0.0
    )
    return neg + pos


def kernel(**inputs):
    pred = np.asarray(inputs["pred"], dtype=np.float32)
    gt = np.asarray(inputs["gt"], dtype=np.float32)
    output = np.asarray(inputs["output"], dtype=np.float32)
    mask = np.asarray(inputs["mask"])
    ind = np.asarray(inputs["ind"]).astype(np.int64)
    target = np.asarray(inputs["target"], dtype=np.float32)
    inde = np.asarray(inputs["inde"]).astype(np.int64)

    b, c_out = output.shape[0], output.shape[1]
    k = ind.shape[1]

    # ---- device: bulk focal reduction at unmodified pred -------------------
    if float(pred.max()) >= 1.0:
        # Out-of-distribution input (spec: uniform [0,1)); the device path
        # computes ln(1-p) without eps, which only differs when p >= 1.
        S = _host_focal_sum(pred, gt)
    else:
        S = float(sum(_device_focal_sums(pred, gt)))

    # ---- host: gather + smooth-L1 + vals (fp64) ----------------------------
    o2 = output.reshape(b, c_out, -1).astype(np.float64)
    pre = np.stack(
        [np.take_along_axis(o2[:, c, :], ind, axis=1) for c in range(c_out)], axis=2
    )  # [B,K,CREG]
    d = pre - target.astype(np.float64)
    ad = np.abs(d)
    huber = np.where(ad < 1.0, 0.5 * d * d, ad - 0.5)
    l_bk = huber.mean(axis=2)  # [B,K]

    pos_mask = mask.astype(bool)
    factor = np.arctan(l_bk) * (2.0 / np.pi)
    vals = np.where(pos_mask, factor, 1.0)  # [B,K]

    # loss0: smooth-L1 of the last positive in flat (b,k) order
    flat_m = pos_mask.reshape(-1)
    nz = np.nonzero(flat_m)[0]
    loss0 = float(l_bk.reshape(-1)[nz[-1]]) if nz.size else 0.0

    # ---- host: multiplicative scatter + focal corrections ------------------
    b_idx = np.broadcast_to(np.arange(b)[:, None], (b, k)).reshape(-1)
    ch = inde[..., 0].reshape(-1)
    yy = inde[..., 1].reshape(-1)
    xx = inde[..., 2].reshape(-1)
    u = ((b_idx * NCLS + ch) * H + yy) * W + xx  # flat positions into pred
    uu, invmap = np.unique(u, return_inverse=True)
    prod = np.ones(uu.size, dtype=np.float64)
    np.multiply.at(prod, invmap, vals.reshape(-1))

    p_old = pred.reshape(-1)[uu].astype(np.float64)
    p_new = p_old * prod
    gtv_u = gt.reshape(-1)[uu].astype(np.float64)
    g_u = GAMMAS[(uu // (H * W)) % NCLS]
    w_u = g_u * 0.5
    delta = float(
        np.sum(w_u * (_focal_terms(p_new, gtv_u, g_u) - _focal_terms(p_old, gtv_u, g_u)))
    )

    # ---- host: positives (gt == 1.0) — vanishing probability path ----------
    num_pos = 0
    pos_total = 0.0
    if float(gt.max()) >= 1.0:
        pm = gt == np.float32(1.0)
        num_pos = int(pm.sum())
        if num_pos:
            pw = np.where(pm)
            pvals = pred[pw].astype(np.float64)
            gpos = GAMMAS[pw[1]]
            pos_total = float(
                np.sum(gpos * 0.5 * np.log(pvals + EPS) * np.power(1.0 - pvals, gpos))
            )

    loss = loss0 - (S + pos_total + delta)
    if num_pos > 0:
        loss = loss / num_pos
    return np.asarray(np.float32(loss))
